# revision 2
# baseline (speedup 1.0000x reference)
"""GCN message-passing kernel for 8 Trainium2 NeuronCores.

Strategy: shard destination nodes across cores (6272 rows/core). Each core
aggregates all edges targeting its rows via gather (dma_gather from a
replicated bf16 node-feature table) + selector-matmul segment-sum in PSUM.
Layer tables (h = x @ W, row-scaled by deg^-1/2) are computed per-slice and
AllGathered. SE attention + 1x1 conv are tiny and computed replicated.
"""
import os
import sys

sys.path.insert(0, "/opt/trn_rl_repo")

from contextlib import ExitStack

import numpy as np

import concourse.bacc as bacc
import concourse.tile as tile
from concourse.tile import add_dep_helper
from concourse import mybir
from concourse.bass_utils import run_bass_kernel_spmd

N = 50000
FM = 128
E = 800000
NCORES = 8
NPOS = 49                  # 128-row tiles per core
RPC = NPOS * 128           # 6272 rows per core
NPAD = NCORES * RPC        # 50176
HALF = NPAD // 2           # 25088 (int16 gather index limit per table half)
SG = 4                     # positions per gather supergroup
VIEWS = ("f", "s", "g")

f32 = mybir.dt.float32
bf16 = mybir.dt.bfloat16
i16 = mybir.dt.int16

_last_exec_time_ns = None


def _split_multiwaits(nc):
    """This walrus build accepts only ONE sync-wait per instruction; split
    extras into preceding same-engine single-wait NoOps (sequencer executes
    waits in program order, so semantics are preserved)."""
    n = 0
    for fn in nc.m.functions:
        for bb in fn.blocks:
            newlist = []
            for inst in bb.instructions:
                si = inst.sync_info
                if si is not None and len(si.on_wait) > 1:
                    waits = list(si.on_wait)
                    for w in waits[:-1]:
                        nop = mybir.InstNoOp(name=f"WSPL-{nc.next_id()}", ins=[], outs=[])
                        nop.engine = inst.engine
                        nop.sync_info = mybir.SyncInfo(on_wait=[w], on_update=[])
                        newlist.append(nop)
                        n += 1
                    si.on_wait = [waits[-1]]
                newlist.append(inst)
            bb.instructions = newlist
    return n


def _prep_view(edges, ew):
    """Host edge preprocessing for one view: append self-loops, compute the
    symmetric GCN normalization, shard by destination across cores, group by
    (dst tile, src half), pad runs to 128-edge chunks (uniform across cores).

    Returns (idx_arrs, dstrel_arrs, w_arrs, NLO, NHI) where the arrays are
    per-core and NLO/NHI are the global per-position chunk counts."""
    src = np.concatenate([edges[0], np.arange(N, dtype=np.int64)])
    dst = np.concatenate([edges[1], np.arange(N, dtype=np.int64)])
    w = np.concatenate([ew.astype(np.float64), np.ones(N)])
    deg = np.bincount(dst, weights=w, minlength=N)
    dis = 1.0 / np.sqrt(deg)
    norm = (dis[src] * w * dis[dst]).astype(np.float32)

    core = dst // RPC
    pos = (dst % RPC) // 128
    dstrel = (dst % 128).astype(np.float32)
    half = (src >= HALF).astype(np.int64)
    idx = (src - HALF * half).astype(np.int16)

    # counts[c, p, h]
    key = (core * NPOS + pos) * 2 + half
    counts = np.bincount(key, minlength=NCORES * NPOS * 2).reshape(NCORES, NPOS, 2)
    chunks = -(-counts // 128)  # ceil
    NLO = chunks[:, :, 0].max(axis=0)
    NHI = chunks[:, :, 1].max(axis=0)

    order = np.lexsort((half, pos, core))
    src_s, norm_s, dstrel_s, idx_s, key_s = (
        src[order], norm[order], dstrel[order], idx[order], key[order])
    starts = np.searchsorted(key_s, np.arange(NCORES * NPOS * 2))
    ends = np.searchsorted(key_s, np.arange(NCORES * NPOS * 2), side="right")

    C = int((NLO + NHI).sum())
    idx_arrs, dr_arrs, w_arrs = [], [], []
    for c in range(NCORES):
        idx_a = np.zeros(C * 128, np.int16)
        dr_a = np.zeros(C * 128, np.float32)
        w_a = np.zeros(C * 128, np.float32)
        off = 0
        sgs = [list(range(s, min(s + SG, NPOS))) for s in range(0, NPOS, SG)]
        for sg in sgs:
            for h in range(2):
                for p in sg:
                    k = (c * NPOS + p) * 2 + h
                    s0, e0 = starts[k], ends[k]
                    n = e0 - s0
                    nch = (NLO if h == 0 else NHI)[p]
                    idx_a[off:off + n] = idx_s[s0:e0]
                    dr_a[off:off + n] = dstrel_s[s0:e0]
                    w_a[off:off + n] = norm_s[s0:e0]
                    off += nch * 128
        assert off == C * 128
        # device layouts
        idx_wrapped = np.tile(idx_a.reshape(-1, 16).T, (8, 1)).copy()  # [128, C*8]
        dr2 = dr_a.reshape(C, 128).T.copy()  # [128, C]
        w2 = w_a.reshape(C, 128).T.copy()
        idx_arrs.append(idx_wrapped)
        dr_arrs.append(dr2)
        w_arrs.append(w2)
    return idx_arrs, dr_arrs, w_arrs, NLO.astype(int), NHI.astype(int)


def _build(meta):
    """Build the SPMD program. meta[v] = (NLO, NHI, C) per view."""
    nc = bacc.Bacc("TRN2", target_bir_lowering=False, debug=False,
                   num_devices=NCORES)

    # ---- I/O ----
    xT_in = nc.dram_tensor("xT_slice", [128, RPC], f32, kind="ExternalInput").ap()
    W_in, b_in = {}, {}
    for nm in ["f1", "f2", "s1", "s2", "g1", "g2"]:
        W_in[nm] = nc.dram_tensor(f"W_{nm}", [FM, FM], f32, kind="ExternalInput").ap()
        b_in[nm] = nc.dram_tensor(f"b_{nm}", [FM], f32, kind="ExternalInput").ap()
    idx_in, dr_in, w_in = {}, {}, {}
    for v in VIEWS:
        C = meta[v][2]
        idx_in[v] = nc.dram_tensor(f"idx_{v}", [128, C * 8], i16, kind="ExternalInput").ap()
        dr_in[v] = nc.dram_tensor(f"dr_{v}", [128, C], f32, kind="ExternalInput").ap()
        w_in[v] = nc.dram_tensor(f"w_{v}", [128, C], f32, kind="ExternalInput").ap()
    mask_in = nc.dram_tensor("mask", [128, NPOS], f32, kind="ExternalInput").ap()
    iota_in = nc.dram_tensor("iota", [128, 128], f32, kind="ExternalInput").ap()
    ident_in = nc.dram_tensor("ident", [128, 128], f32, kind="ExternalInput").ap()
    fc1wT_in = nc.dram_tensor("fc1wT", [6, 30], f32, kind="ExternalInput").ap()
    fc1b_in = nc.dram_tensor("fc1b", [30], f32, kind="ExternalInput").ap()
    fc2wT_in = nc.dram_tensor("fc2wT", [30, 6], f32, kind="ExternalInput").ap()
    fc2b_in = nc.dram_tensor("fc2b", [6], f32, kind="ExternalInput").ap()
    cnnw_in = nc.dram_tensor("cnnw", [6], f32, kind="ExternalInput").ap()
    cnnb_in = nc.dram_tensor("cnnb", [1], f32, kind="ExternalInput").ap()
    out_d = nc.dram_tensor("out_slice", [RPC, FM], f32, kind="ExternalOutput").ap()

    with tile.TileContext(nc) as tc, ExitStack() as ctx:
        singles = ctx.enter_context(tc.tile_pool(name="singles", bufs=1))
        pool = ctx.enter_context(tc.tile_pool(name="pool", bufs=3))
        selp = ctx.enter_context(tc.tile_pool(name="selp", bufs=4))
        gpo = ctx.enter_context(tc.tile_pool(name="gpo", bufs=2))
        fpo = ctx.enter_context(tc.tile_pool(name="fpo", bufs=3))
        psA = ctx.enter_context(tc.tile_pool(name="psA", bufs=4, space="PSUM"))
        psB = ctx.enter_context(tc.tile_pool(name="psB", bufs=4, space="PSUM"))
        dram = ctx.enter_context(tc.tile_pool(name="dram", bufs=1, space="DRAM"))

        table = dram.tile([NPAD, FM], bf16, name="table")
        tab_slice = dram.tile([RPC, FM], bf16, name="tab_slice")
        f_sl = [dram.tile([RPC, FM], f32, name=f"f_sl{i}") for i in range(6)]
        pool6_in = dram.tile([6, 1], f32, name="pool6_in")
        pool6_out = dram.tile([6, 1], f32, name="pool6_out")
        a_scr = dram.tile([6], f32, name="a_scr")

        # ---- constants ----
        iota_f = singles.tile([128, 128], f32)
        nc.sync.dma_start(out=iota_f[:], in_=iota_in[:])
        ident = singles.tile([128, 128], f32)
        nc.sync.dma_start(out=ident[:], in_=ident_in[:])
        mask_sb = singles.tile([128, NPOS], f32)
        nc.sync.dma_start(out=mask_sb[:], in_=mask_in[:])
        W_sb, bb_sb = {}, {}
        for nm in ["f1", "f2", "s1", "s2", "g1", "g2"]:
            W_sb[nm] = singles.tile([FM, FM], f32, tag=f"W_{nm}", name=f"Wsb_{nm}")
            nc.sync.dma_start(out=W_sb[nm][:], in_=W_in[nm][:])
            bb_sb[nm] = singles.tile([128, FM], f32, tag=f"bb_{nm}", name=f"bbsb_{nm}")
            nc.gpsimd.dma_start(out=bb_sb[nm][:], in_=b_in[nm].partition_broadcast(128))
        pooled_acc = singles.tile([128, 6], f32)
        nc.vector.memset(pooled_acc[:], 0.0)

        idx_sb, dr_sb, w_sb = {}, {}, {}
        for v in VIEWS:
            C = meta[v][2]
            idx_sb[v] = singles.tile([128, C * 8], i16, tag=f"idx_{v}", name=f"idxsb_{v}")
            nc.sync.dma_start(out=idx_sb[v][:], in_=idx_in[v][:])
            dr_sb[v] = singles.tile([128, C], f32, tag=f"dr_{v}", name=f"drsb_{v}")
            nc.sync.dma_start(out=dr_sb[v][:], in_=dr_in[v][:])
            w_sb[v] = singles.tile([128, C], f32, tag=f"w_{v}", name=f"wsb_{v}")
            nc.sync.dma_start(out=w_sb[v][:], in_=w_in[v][:])

        def tab_phase(src_kind, vsrc_l, Wn):
            """Compute tab_slice = cast_bf16((src @ W)) for own rows.
            src_kind 'x': from xT input (already transposed);
            src_kind 'f': from f_sl[vsrc_l] (needs transpose first)."""
            for p in range(NPOS):
                if src_kind == "x":
                    t_fn = pool.tile([128, 128], f32, tag="tabin")
                    nc.sync.dma_start(out=t_fn[:], in_=xT_in[:, p * 128:(p + 1) * 128])
                else:
                    t_nf = pool.tile([128, 128], f32, tag="tabin")
                    nc.sync.dma_start(out=t_nf[:], in_=f_sl[vsrc_l][p * 128:(p + 1) * 128, :])
                    ptr = psB.tile([128, 128], f32, tag="tabps")
                    nc.tensor.transpose(out=ptr[:], in_=t_nf[:], identity=ident[:])
                    t_fn = pool.tile([128, 128], f32, tag="tabin2")
                    nc.vector.tensor_copy(out=t_fn[:], in_=ptr[:])
                pm = psB.tile([128, 128], f32, tag="tabps")
                nc.tensor.matmul(pm[:], lhsT=W_sb[Wn][:], rhs=t_fn[:], start=True, stop=True)
                tmid = pool.tile([128, 128], f32, tag="tmid")
                nc.vector.tensor_copy(out=tmid[:], in_=pm[:])
                ptr2 = psB.tile([128, 128], f32, tag="tabps")
                nc.tensor.transpose(out=ptr2[:], in_=tmid[:], identity=ident[:])
                tb = pool.tile([128, 128], bf16, tag="tbf")
                nc.vector.tensor_copy(out=tb[:], in_=ptr2[:])
                nc.sync.dma_start(out=tab_slice[p * 128:(p + 1) * 128, :], in_=tb[:])

        state = {"last_ag": None, "gathers": []}

        def allgather_table():
            ag = nc.gpsimd.collective_compute(
                "AllGather", mybir.AluOpType.bypass,
                replica_groups=[list(range(NCORES))],
                ins=[tab_slice[:]], outs=[table[:]],
            )
            # serialize table reuse: AllGather must wait for all gathers of
            # the previous layer (WAR), and subsequent gathers must wait
            # for this AllGather (RAW) — custom-DMA APs are not reliably
            # dep-tracked by Tile.
            for g in state["gathers"]:
                add_dep_helper(ag.ins, g.ins, reason="table WAR")
            state["gathers"] = []
            state["last_ag"] = ag

        def agg_phase(v, Wn, l_out):
            NLO, NHI, C = meta[v]
            bname = Wn
            sgs = [list(range(s, min(s + SG, NPOS))) for s in range(0, NPOS, SG)]
            chunk_base = 0  # global chunk counter into dr/w arrays
            idx_col = 0     # column offset into idx_sb (units of 16 idxs)
            for sg in sgs:
                nlo = int(sum(NLO[p] for p in sg))
                nhi = int(sum(NHI[p] for p in sg))
                glo = gpo.tile([128, max(nlo, 1), 128], bf16, tag="glo")
                ghi = gpo.tile([128, max(nhi, 1), 128], bf16, tag="ghi")
                GMAXC = 8  # chunks per dma_gather (1024 idxs; >=2048 hangs SWDGE)
                for g0 in range(0, nlo, GMAXC):
                    gn = min(GMAXC, nlo - g0)
                    gi = nc.gpsimd.dma_gather(
                        out_ap=glo[:, g0:g0 + gn, :], in_ap=table[0:HALF, :],
                        idxs_ap=idx_sb[v][:, idx_col:idx_col + gn * 8],
                        num_idxs=gn * 128, num_idxs_reg=gn * 128, elem_size=128,
                    )
                    add_dep_helper(gi.ins, state["last_ag"].ins, reason="table RAW")
                    state["gathers"].append(gi)
                    idx_col += gn * 8
                for g0 in range(0, nhi, GMAXC):
                    gn = min(GMAXC, nhi - g0)
                    gi = nc.gpsimd.dma_gather(
                        out_ap=ghi[:, g0:g0 + gn, :], in_ap=table[HALF:NPAD, :],
                        idxs_ap=idx_sb[v][:, idx_col:idx_col + gn * 8],
                        num_idxs=gn * 128, num_idxs_reg=gn * 128, elem_size=128,
                    )
                    add_dep_helper(gi.ins, state["last_ag"].ins, reason="table RAW")
                    state["gathers"].append(gi)
                    idx_col += gn * 8
                # chunk order in dr/w arrays: [lo(p0)..lo(pk)] then [hi(p0)..hi(pk)]
                lo_off = {}
                off = 0
                for p in sg:
                    lo_off[p] = off
                    off += int(NLO[p])
                hi_off = {}
                off = 0
                for p in sg:
                    hi_off[p] = off
                    off += int(NHI[p])
                psums = {}
                for p in sg:
                    ps = psA.tile([128, 128], f32, tag="agg")
                    psums[p] = ps
                    nch = int(NLO[p] + NHI[p])
                    ci = 0
                    for k in range(int(NLO[p])):
                        cg = chunk_base + lo_off[p] + k
                        sel = selp.tile([128, 128], bf16, tag="sel")
                        nc.vector.tensor_scalar(
                            out=sel[:], in0=iota_f[:],
                            scalar1=dr_sb[v][:, cg:cg + 1], scalar2=w_sb[v][:, cg:cg + 1],
                            op0=mybir.AluOpType.is_equal, op1=mybir.AluOpType.mult)
                        nc.tensor.matmul(ps[:], lhsT=sel[:], rhs=glo[:, lo_off[p] + k, :],
                                         start=(ci == 0), stop=(ci == nch - 1))
                        ci += 1
                    for k in range(int(NHI[p])):
                        cg = chunk_base + nlo + hi_off[p] + k
                        sel = selp.tile([128, 128], bf16, tag="sel")
                        nc.vector.tensor_scalar(
                            out=sel[:], in0=iota_f[:],
                            scalar1=dr_sb[v][:, cg:cg + 1], scalar2=w_sb[v][:, cg:cg + 1],
                            op0=mybir.AluOpType.is_equal, op1=mybir.AluOpType.mult)
                        nc.tensor.matmul(ps[:], lhsT=sel[:], rhs=ghi[:, hi_off[p] + k, :],
                                         start=(ci == 0), stop=(ci == nch - 1))
                        ci += 1
                    # postprocess: f = mask * relu(agg + b)
                    ft = fpo.tile([128, 128], f32, tag="ftile")
                    nc.vector.tensor_tensor(out=ft[:], in0=ps[:], in1=bb_sb[bname][:],
                                            op=mybir.AluOpType.add)
                    nc.vector.tensor_scalar_max(ft[:], ft[:], 0.0)
                    nc.vector.tensor_scalar_mul(ft[:], ft[:], mask_sb[:, p:p + 1])
                    r = fpo.tile([128, 1], f32, tag="fred")
                    nc.vector.tensor_reduce(out=r[:], in_=ft[:], axis=mybir.AxisListType.X,
                                            op=mybir.AluOpType.add)
                    nc.vector.tensor_tensor(out=pooled_acc[:, l_out:l_out + 1],
                                            in0=pooled_acc[:, l_out:l_out + 1], in1=r[:],
                                            op=mybir.AluOpType.add)
                    nc.sync.dma_start(out=f_sl[l_out][p * 128:(p + 1) * 128, :], in_=ft[:])
                chunk_base += nlo + nhi

        scope = os.environ.get("KERNEL_SCOPE", "full")
        if scope == "tab":
            tab_phase("x", None, "f1")
        elif scope == "tabag":
            tab_phase("x", None, "f1")
            allgather_table()
        elif scope == "agg1":
            tab_phase("x", None, "f1")
            allgather_table()
            agg_phase("f", "f1", 0)
        views_iter = VIEWS if scope == "full" else ()
        for vi, v in enumerate(views_iter):
            n1, n2 = f"{v}1", f"{v}2"
            tab_phase("x", None, n1)
            allgather_table()
            agg_phase(v, n1, 2 * vi)
            tab_phase("f", 2 * vi, n2)
            allgather_table()
            agg_phase(v, n2, 2 * vi + 1)

        # ---- pooled -> SE attention scalars ----
        for l in range(6):
            rc = fpo.tile([1, 1], f32, tag="rc", name=f"rc{l}")
            nc.gpsimd.tensor_reduce(out=rc[:], in_=pooled_acc[:, l:l + 1],
                                    axis=mybir.AxisListType.C, op=mybir.AluOpType.add)
            nc.sync.dma_start(out=pool6_in[l:l + 1, :], in_=rc[:])
        nc.gpsimd.collective_compute(
            "AllReduce", mybir.AluOpType.add,
            replica_groups=[list(range(NCORES))],
            ins=[pool6_in[:]], outs=[pool6_out[:]],
        )
        pvec2 = singles.tile([6, 1], f32)
        nc.sync.dma_start(out=pvec2[:], in_=pool6_out[:])
        nc.vector.tensor_scalar_mul(pvec2[:], pvec2[:], 1.0 / (N * FM))
        fc1wT = singles.tile([6, 30], f32)
        nc.sync.dma_start(out=fc1wT[:], in_=fc1wT_in[:])
        fc1b = singles.tile([30, 1], f32)
        nc.sync.dma_start(out=fc1b[:], in_=fc1b_in.unsqueeze(1))
        fc2wT = singles.tile([30, 6], f32)
        nc.sync.dma_start(out=fc2wT[:], in_=fc2wT_in[:])
        fc2b = singles.tile([6, 1], f32)
        nc.sync.dma_start(out=fc2b[:], in_=fc2b_in.unsqueeze(1))
        pz1 = psB.tile([30, 1], f32, tag="tabps")
        nc.tensor.matmul(pz1[:], lhsT=fc1wT[:], rhs=pvec2[:], start=True, stop=True)
        z1 = singles.tile([30, 1], f32)
        nc.vector.tensor_tensor(out=z1[:], in0=pz1[:], in1=fc1b[:], op=mybir.AluOpType.add)
        nc.vector.tensor_scalar_max(z1[:], z1[:], 0.0)
        pz2 = psB.tile([6, 1], f32, tag="tabps")
        nc.tensor.matmul(pz2[:], lhsT=fc2wT[:], rhs=z1[:], start=True, stop=True)
        z2 = singles.tile([6, 1], f32)
        nc.vector.tensor_tensor(out=z2[:], in0=pz2[:], in1=fc2b[:], op=mybir.AluOpType.add)
        av = singles.tile([6, 1], f32)
        nc.scalar.activation(out=av[:], in_=z2[:], func=mybir.ActivationFunctionType.Sigmoid)
        nc.sync.dma_start(out=a_scr[:], in_=av[:, 0])
        a_b = singles.tile([128, 6], f32)
        nc.gpsimd.dma_start(out=a_b[:], in_=a_scr[:].partition_broadcast(128))
        cnnw_b = singles.tile([128, 6], f32)
        nc.gpsimd.dma_start(out=cnnw_b[:], in_=cnnw_in.partition_broadcast(128))
        cnnb_b = singles.tile([128, 1], f32)
        nc.gpsimd.dma_start(out=cnnb_b[:], in_=cnnb_in.partition_broadcast(128))

        # ---- final combine: out = sum_l cnnw_l * relu(a_l * f_l) + cnn_b ----
        if scope != "full":
            for p in range(NPOS):
                fl0 = fpo.tile([128, 128], f32, tag="fin", name=f"fl0_{p}")
                nc.sync.dma_start(out=fl0[:], in_=f_sl[0][p * 128:(p + 1) * 128, :])
                nc.sync.dma_start(out=out_d[p * 128:(p + 1) * 128, :], in_=fl0[:])
        for p in range(NPOS) if scope == "full" else []:
            acc = fpo.tile([128, 128], f32, tag="facc")
            for l in range(6):
                fl = fpo.tile([128, 128], f32, tag="fin")
                nc.sync.dma_start(out=fl[:], in_=f_sl[l][p * 128:(p + 1) * 128, :])
                t = fpo.tile([128, 128], f32, tag="ftmp")
                nc.vector.tensor_scalar_mul(t[:], fl[:], a_b[:, l:l + 1])
                nc.vector.tensor_scalar_max(t[:], t[:], 0.0)
                nc.vector.tensor_scalar_mul(t[:], t[:], cnnw_b[:, l:l + 1])
                if l == 0:
                    nc.vector.tensor_copy(out=acc[:], in_=t[:])
                else:
                    nc.vector.tensor_tensor(out=acc[:], in0=acc[:], in1=t[:],
                                            op=mybir.AluOpType.add)
            nc.vector.tensor_scalar_add(acc[:], acc[:], cnnb_b[:, 0:1])
            nc.sync.dma_start(out=out_d[p * 128:(p + 1) * 128, :], in_=acc[:])

    nc.compile()
    _split_multiwaits(nc)
    return nc


def kernel(**inputs):
    global _last_exec_time_ns
    inputs = {k: np.asarray(v) for k, v in inputs.items()}

    meta = {}
    perview = {}
    for v in VIEWS:
        idx_arrs, dr_arrs, w_arrs, NLO, NHI = _prep_view(
            inputs[f"edges_{v}"].astype(np.int64), inputs[f"ew_{v}"])
        meta[v] = (NLO, NHI, int((NLO + NHI).sum()))
        perview[v] = (idx_arrs, dr_arrs, w_arrs)

    nc = _build(meta)

    xT = inputs["x_m"].T.astype(np.float32)  # [128, N]
    xT_pad = np.zeros((128, NPAD), np.float32)
    xT_pad[:, :N] = xT
    node_ids = np.arange(NPAD)
    mask_full = (node_ids < N).astype(np.float32)
    iota_np = np.broadcast_to(np.arange(128, dtype=np.float32), (128, 128)).copy()
    ident_np = np.eye(128, dtype=np.float32)

    in_maps = []
    for c in range(NCORES):
        m = {
            "xT_slice": np.ascontiguousarray(xT_pad[:, c * RPC:(c + 1) * RPC]),
            "mask": np.ascontiguousarray(
                mask_full[c * RPC:(c + 1) * RPC].reshape(NPOS, 128).T),
            "iota": iota_np,
            "ident": ident_np,
            "fc1wT": inputs["fc1_w"].T.astype(np.float32).copy(),
            "fc1b": inputs["fc1_b"].astype(np.float32),
            "fc2wT": inputs["fc2_w"].T.astype(np.float32).copy(),
            "fc2b": inputs["fc2_b"].astype(np.float32),
            "cnnw": inputs["cnn_w"].astype(np.float32),
            "cnnb": inputs["cnn_b"].astype(np.float32),
        }
        for nm in ["f1", "f2", "s1", "s2", "g1", "g2"]:
            m[f"W_{nm}"] = inputs[f"W_{nm}"].astype(np.float32)
            m[f"b_{nm}"] = inputs[f"b_{nm}"].astype(np.float32)
        for v in VIEWS:
            idx_arrs, dr_arrs, w_arrs = perview[v]
            m[f"idx_{v}"] = idx_arrs[c]
            m[f"dr_{v}"] = dr_arrs[c]
            m[f"w_{v}"] = w_arrs[c]
        in_maps.append(m)

    trace = os.environ.get("KERNEL_TRACE", "0") == "1"
    kw = {}
    if trace:
        td = os.environ.get("KERNEL_TRACE_DIR")
        if td:
            os.makedirs(td, exist_ok=True)
            kw["tmpdir"] = td
    res = run_bass_kernel_spmd(nc, in_maps, list(range(NCORES)), trace=trace, **kw)
    _last_exec_time_ns = res.exec_time_ns
    out = np.concatenate([res.results[c]["out_slice"] for c in range(NCORES)], axis=0)
    return out[:N].astype(np.float32)



# revision 15
# speedup vs baseline: 1.1771x; 1.1771x over previous
"""GCN message-passing kernel for 8 Trainium2 NeuronCores.

Strategy: shard destination nodes across cores (6272 rows/core). Each core
aggregates all edges targeting its rows by gathering source rows from a
replicated bf16 node-feature table (SWDGE dma_gather, prepare_only +
trigger_dma so the Pool engine pipelines descriptor-gen with transfers) and
contracting each 128-edge chunk against a host-precomputed one-hot selector
(streamed from HBM) on the PE array. The aggregation runs transposed
(psum[feat, dst]) so bias+relu+row-sum fuse into one Activation-engine op.
Layer tables ping-pong between two DRAM buffers so each AllGather overlaps
the previous layer's aggregation. SE attention + 1x1 conv are tiny and
replicated; the final output is produced transposed and fixed up on host.
"""
import os
import sys

sys.path.insert(0, "/opt/trn_rl_repo")

from contextlib import ExitStack

import ml_dtypes
import numpy as np

import concourse.bacc as bacc
import concourse.tile as tile
from concourse.tile import add_dep_helper
from concourse import bass_isa, mybir
from concourse.bass_utils import run_bass_kernel_spmd

N = 50000
FM = 128
E = 800000
NCORES = 8
NPOS = 49                  # 128-row tiles per core
RPC = NPOS * 128           # 6272 rows per core
NPAD = NCORES * RPC        # 50176
HALF = NPAD // 2           # 25088 (int16 gather index limit per table half)
SG = 4                     # positions per gather supergroup
VIEWS = ("f", "s", "g")
LAYERS = [("f", 1), ("s", 1), ("g", 1), ("f", 2), ("s", 2), ("g", 2)]

f32 = mybir.dt.float32
bf16 = mybir.dt.bfloat16
i16 = mybir.dt.int16
bfnp = ml_dtypes.bfloat16

_last_exec_time_ns = None


def _split_multiwaits(nc):
    """This walrus build accepts only ONE sync-wait per instruction; split
    extras into preceding same-engine single-wait NoOps (sequencer executes
    waits in program order, so semantics are preserved)."""
    n = 0
    for fn in nc.m.functions:
        for bb in fn.blocks:
            newlist = []
            for inst in bb.instructions:
                si = inst.sync_info
                if si is not None and len(si.on_wait) > 1:
                    waits = list(si.on_wait)
                    for w in waits[:-1]:
                        nop = mybir.InstNoOp(name=f"WSPL-{nc.next_id()}", ins=[], outs=[])
                        nop.engine = inst.engine
                        nop.sync_info = mybir.SyncInfo(on_wait=[w], on_update=[])
                        newlist.append(nop)
                        n += 1
                    si.on_wait = [waits[-1]]
                newlist.append(inst)
            bb.instructions = newlist
    return n


def _prep_view(edges, ew):
    """Host edge preprocessing for one view: append self-loops, compute the
    symmetric GCN normalization, shard by destination across cores, group by
    (dst tile, src half), pad runs to 128-edge chunks (uniform across cores).

    Returns (idx_arrs, sel_arrs, NLO, NHI): per-core SWDGE index arrays and
    precomputed one-hot selector chunks ([128 edge-slot partitions, C*128
    dst columns], bf16, selector value = the edge's GCN norm weight)."""
    src = np.concatenate([edges[0], np.arange(N, dtype=np.int64)])
    dst = np.concatenate([edges[1], np.arange(N, dtype=np.int64)])
    w = np.concatenate([ew.astype(np.float64), np.ones(N)])
    deg = np.bincount(dst, weights=w, minlength=N)
    dis = 1.0 / np.sqrt(deg)
    norm = (dis[src] * w * dis[dst]).astype(np.float32)

    core = dst // RPC
    pos = (dst % RPC) // 128
    dstrel = (dst % 128).astype(np.int64)
    half = (src >= HALF).astype(np.int64)
    idx = (src - HALF * half).astype(np.int16)

    # counts[c, p, h]
    key = (core * NPOS + pos) * 2 + half
    counts = np.bincount(key, minlength=NCORES * NPOS * 2).reshape(NCORES, NPOS, 2)
    chunks = -(-counts // 128)  # ceil
    NLO = chunks[:, :, 0].max(axis=0)
    NHI = chunks[:, :, 1].max(axis=0)

    order = np.lexsort((half, pos, core))
    norm_s, dstrel_s, idx_s, key_s = (
        norm[order], dstrel[order], idx[order], key[order])
    starts = np.searchsorted(key_s, np.arange(NCORES * NPOS * 2))
    ends = np.searchsorted(key_s, np.arange(NCORES * NPOS * 2), side="right")

    C = int((NLO + NHI).sum())
    idx_arrs, sel_arrs = [], []
    sgs = [list(range(s, min(s + SG, NPOS))) for s in range(0, NPOS, SG)]
    for c in range(NCORES):
        idx_a = np.zeros(C * 128, np.int16)
        dr_a = np.zeros(C * 128, np.int64)
        w_a = np.zeros(C * 128, np.float32)
        off = 0
        for sg in sgs:
            for h in range(2):
                for p in sg:
                    k = (c * NPOS + p) * 2 + h
                    s0, e0 = starts[k], ends[k]
                    n = e0 - s0
                    nch = (NLO if h == 0 else NHI)[p]
                    idx_a[off:off + n] = idx_s[s0:e0]
                    dr_a[off:off + n] = dstrel_s[s0:e0]
                    w_a[off:off + n] = norm_s[s0:e0]
                    off += nch * 128
        assert off == C * 128
        # device layouts
        idx_wrapped = np.tile(idx_a.reshape(-1, 16).T, (8, 1)).copy()  # [128, C*8]
        sel_flat = np.zeros((C * 128, 128), np.float32)
        sel_flat[np.arange(C * 128), dr_a] = w_a
        sel_dev = np.ascontiguousarray(
            sel_flat.reshape(C, 128, 128).transpose(1, 0, 2).reshape(128, C * 128)
        ).astype(bfnp)
        idx_arrs.append(idx_wrapped)
        sel_arrs.append(sel_dev)
    return idx_arrs, sel_arrs, NLO.astype(int), NHI.astype(int)


def _build(meta):
    """Build the SPMD program. meta[v] = (NLO, NHI, C) per view."""
    nc = bacc.Bacc("TRN2", target_bir_lowering=False, debug=False,
                   num_devices=NCORES)

    # ---- I/O ----
    xT_in = nc.dram_tensor("xT_slice", [128, RPC], bf16, kind="ExternalInput").ap()
    W_in, b_in = {}, {}
    for nm in ["f1", "f2", "s1", "s2", "g1", "g2"]:
        W_in[nm] = nc.dram_tensor(f"W_{nm}", [FM, FM], bf16, kind="ExternalInput").ap()
        b_in[nm] = nc.dram_tensor(f"b_{nm}", [FM], f32, kind="ExternalInput").ap()
    idx_in, sel_in = {}, {}
    for v in VIEWS:
        C = meta[v][2]
        idx_in[v] = nc.dram_tensor(f"idx_{v}", [128, C * 8], i16, kind="ExternalInput").ap()
        sel_in[v] = nc.dram_tensor(f"sel_{v}", [128, C * 128], bf16, kind="ExternalInput").ap()
    ident_in = nc.dram_tensor("ident", [128, 128], bf16, kind="ExternalInput").ap()
    fc1wT_in = nc.dram_tensor("fc1wT", [6, 30], f32, kind="ExternalInput").ap()
    fc1b_in = nc.dram_tensor("fc1b", [30], f32, kind="ExternalInput").ap()
    fc2wT_in = nc.dram_tensor("fc2wT", [30, 6], f32, kind="ExternalInput").ap()
    fc2b_in = nc.dram_tensor("fc2b", [6], f32, kind="ExternalInput").ap()
    cnnw_in = nc.dram_tensor("cnnw", [6], f32, kind="ExternalInput").ap()
    cnnb_in = nc.dram_tensor("cnnb", [1], f32, kind="ExternalInput").ap()
    corr_in = nc.dram_tensor("corr", [6], f32, kind="ExternalInput").ap()
    out_d = nc.dram_tensor("out_slice", [FM, RPC], f32, kind="ExternalOutput").ap()

    dma_sem = nc.alloc_semaphore("gather_dma")

    with tile.TileContext(nc) as tc, ExitStack() as ctx:
        singles = ctx.enter_context(tc.tile_pool(name="singles", bufs=1))
        pool = ctx.enter_context(tc.tile_pool(name="pool", bufs=3))
        selp = ctx.enter_context(tc.tile_pool(name="selp", bufs=2))
        gpo = ctx.enter_context(tc.tile_pool(name="gpo", bufs=2))
        fpo = ctx.enter_context(tc.tile_pool(name="fpo", bufs=4))
        psA = ctx.enter_context(tc.tile_pool(name="psA", bufs=4, space="PSUM"))
        psB = ctx.enter_context(tc.tile_pool(name="psB", bufs=2, space="PSUM"))
        dram = ctx.enter_context(tc.tile_pool(name="dram", bufs=1, space="DRAM"))

        tables = [dram.tile([NPAD, FM], bf16, name=f"table{i}") for i in range(2)]
        tab_slices = [dram.tile([RPC, FM], bf16, name=f"tab_slice{i}") for i in range(2)]
        fT_sl = [dram.tile([128, RPC], bf16, name=f"fT_sl{i}") for i in range(6)]
        pool6_in = dram.tile([6, 1], f32, name="pool6_in")
        pool6_out = dram.tile([6, 1], f32, name="pool6_out")
        a_scr = dram.tile([6], f32, name="a_scr")

        # ---- constants ----
        ident = singles.tile([128, 128], bf16)
        nc.sync.dma_start(out=ident[:], in_=ident_in[:])
        identf = singles.tile([128, 128], f32)
        nc.vector.tensor_copy(out=identf[:], in_=ident[:])
        W_sb, bb_sb = {}, {}
        for nm in ["f1", "f2", "s1", "s2", "g1", "g2"]:
            W_sb[nm] = singles.tile([FM, FM], bf16, tag=f"W_{nm}", name=f"Wsb_{nm}")
            nc.sync.dma_start(out=W_sb[nm][:], in_=W_in[nm][:])
            bb_sb[nm] = singles.tile([FM, 1], f32, tag=f"bb_{nm}", name=f"bbsb_{nm}")
            nc.sync.dma_start(out=bb_sb[nm][:], in_=b_in[nm].unsqueeze(1))
        pooled_acc = singles.tile([128, 6], f32)
        nc.vector.memset(pooled_acc[:], 0.0)

        idx_sb = {}
        for v in VIEWS:
            C = meta[v][2]
            idx_sb[v] = singles.tile([128, C * 8], i16, tag=f"idx_{v}", name=f"idxsb_{v}")
            nc.sync.dma_start(out=idx_sb[v][:], in_=idx_in[v][:])

        # per-table-buffer state for manual collective/gather dep tracking
        # (custom-DMA APs over DRAM pool tiles are not reliably dep-tracked)
        tabst = [{"ag": None, "preps": []} for _ in range(2)]
        slice_ag = [None, None]   # last AllGather reading tab_slices[i]

        def tab_phase(src_kind, vsrc_l, Wn, ts):
            """tab_slices[ts] = cast_bf16(src @ W) for own rows.
            src 'x': xT input; src 'f': fT_sl[vsrc_l] (both [feat, node])."""
            war = slice_ag[ts]
            for p in range(NPOS):
                cols = slice(p * 128, (p + 1) * 128)
                t_fn = pool.tile([128, 128], bf16, tag="tabin")
                if src_kind == "x":
                    ld = nc.sync.dma_start(out=t_fn[:], in_=xT_in[:, cols])
                else:
                    ld = nc.sync.dma_start(out=t_fn[:], in_=fT_sl[vsrc_l][:, cols])
                pm = psB.tile([128, 128], f32, tag="tabps")
                nc.tensor.matmul(pm[:], lhsT=W_sb[Wn][:], rhs=t_fn[:], start=True, stop=True)
                tmid = pool.tile([128, 128], f32, tag="tmid")
                nc.scalar.copy(out=tmid[:], in_=pm[:])
                ptr2 = psB.tile([128, 128], f32, tag="tabps2")
                nc.tensor.transpose(out=ptr2[:], in_=tmid[:], identity=identf[:])
                tb = pool.tile([128, 128], bf16, tag="tbf")
                nc.vector.tensor_copy(out=tb[:], in_=ptr2[:])
                st = nc.sync.dma_start(out=tab_slices[ts][p * 128:(p + 1) * 128, :], in_=tb[:])
                if war is not None:
                    add_dep_helper(st.ins, war.ins, reason="tab_slice WAR")
                yield st

        def allgather_table(tab_stores, ts, buf):
            ag = nc.gpsimd.collective_compute(
                "AllGather", mybir.AluOpType.bypass,
                replica_groups=[list(range(NCORES))],
                ins=[tab_slices[ts][:]], outs=[tables[buf][:]],
            )
            for st in tab_stores:
                add_dep_helper(ag.ins, st.ins, reason="tab_slice RAW")
            for g in tabst[buf]["preps"]:
                add_dep_helper(ag.ins, g.ins, reason="table WAR")
            tabst[buf] = {"ag": ag, "preps": []}
            slice_ag[ts] = ag

        def agg_phase(v, Wn, l_out, buf):
            NLO, NHI, C = meta[v]
            ag = tabst[buf]["ag"]
            sgs = [list(range(s, min(s + SG, NPOS))) for s in range(0, NPOS, SG)]
            chunk_base = 0  # global chunk counter
            idx_col = 0     # column offset into idx_sb (units of 16 idxs)
            for sg in sgs:
                nlo = int(sum(NLO[p] for p in sg))
                nhi = int(sum(NHI[p] for p in sg))
                nch_sg = nlo + nhi
                # stream this supergroup's selector chunks (contiguous)
                selsb = selp.tile([128, max(nch_sg, 1) * 128], bf16, tag="sel")
                nc.sync.dma_start(
                    out=selsb[:],
                    in_=sel_in[v][:, chunk_base * 128:(chunk_base + max(nch_sg, 1)) * 128])
                glo = gpo.tile([128, max(nlo, 1), 128], bf16, tag="glo")
                ghi = gpo.tile([128, max(nhi, 1), 128], bf16, tag="ghi")
                GMAXC = 8  # chunks per dma_gather (1024 idxs; >=2048 hangs SWDGE)
                for half_i, (nh, gt, lohi) in enumerate(
                        (((nlo, glo, (0, HALF)), (nhi, ghi, (HALF, NPAD))))):
                    for g0 in range(0, nh, GMAXC):
                        gn = min(GMAXC, nh - g0)
                        gi = nc.gpsimd.dma_gather(
                            out_ap=gt[:, g0:g0 + gn, :],
                            in_ap=tables[buf][lohi[0]:lohi[1], :],
                            idxs_ap=idx_sb[v][:, idx_col:idx_col + gn * 8],
                            num_idxs=gn * 128, num_idxs_reg=gn * 128, elem_size=128,
                        )
                        add_dep_helper(gi.ins, ag.ins, reason="table RAW")
                        tabst[buf]["preps"].append(gi)
                        idx_col += gn * 8
                # chunk order in sel array: [lo(p0)..lo(pk)] then [hi(p0)..hi(pk)]
                lo_off, off = {}, 0
                for p in sg:
                    lo_off[p] = off
                    off += int(NLO[p])
                hi_off, off = {}, 0
                for p in sg:
                    hi_off[p] = off
                    off += int(NHI[p])
                for p in sg:
                    ps = psA.tile([128, 128], f32, tag="agg")
                    nch = int(NLO[p] + NHI[p])
                    ci = 0
                    for k in range(int(NLO[p])):
                        cg = lo_off[p] + k  # sel col block within supergroup
                        nc.tensor.matmul(ps[:], lhsT=glo[:, lo_off[p] + k, :],
                                         rhs=selsb[:, cg * 128:(cg + 1) * 128],
                                         start=(ci == 0), stop=(ci == nch - 1))
                        ci += 1
                    for k in range(int(NHI[p])):
                        cg = nlo + hi_off[p] + k
                        nc.tensor.matmul(ps[:], lhsT=ghi[:, hi_off[p] + k, :],
                                         rhs=selsb[:, cg * 128:(cg + 1) * 128],
                                         start=(ci == 0), stop=(ci == nch - 1))
                        ci += 1
                    # postprocess: fT = relu(agg + b), fused row-sum for pooling
                    ft = fpo.tile([128, 128], bf16, tag="ftile")
                    racc = fpo.tile([128, 1], f32, tag="racc")
                    nc.scalar.activation(
                        out=ft[:], in_=ps[:], func=mybir.ActivationFunctionType.Relu,
                        bias=bb_sb[Wn][:, 0:1], accum_out=racc[:])
                    nc.vector.tensor_tensor(out=pooled_acc[:, l_out:l_out + 1],
                                            in0=pooled_acc[:, l_out:l_out + 1],
                                            in1=racc[:], op=mybir.AluOpType.add)
                    nc.scalar.dma_start(out=fT_sl[l_out][:, p * 128:(p + 1) * 128],
                                        in_=ft[:])
                chunk_base += nch_sg

        scope = os.environ.get("KERNEL_SCOPE", "full")
        if scope == "full":
            # schedule: tab f1, AG f1, tab s1, AG s1, agg f1, tab g1, AG g1,
            # agg s1, tab f2, AG f2, agg g1, ... so each AllGather overlaps
            # the previous layer's aggregation (ping-pong table buffers).
            plans = []
            for i, (v, ln) in enumerate(LAYERS):
                nm = f"{v}{ln}"
                src = ("x", None) if ln == 1 else ("f", 2 * VIEWS.index(v))
                plans.append({"v": v, "nm": nm, "src": src, "l_out": 2 * VIEWS.index(v) + ln - 1,
                              "buf": i % 2, "ts": i % 2})

            def do_tab(i):
                p = plans[i]
                stores = list(tab_phase(p["src"][0], p["src"][1], p["nm"], p["ts"]))
                allgather_table(stores, p["ts"], p["buf"])

            def do_agg(i):
                p = plans[i]
                agg_phase(p["v"], p["nm"], p["l_out"], p["buf"])

            do_tab(0)
            do_tab(1)
            do_agg(0)
            do_tab(2)
            do_agg(1)
            do_tab(3)
            do_agg(2)
            do_tab(4)
            do_agg(3)
            do_tab(5)
            do_agg(4)
            do_agg(5)
        else:
            plans = [{"src": ("x", None), "nm": "f1", "ts": 0, "buf": 0, "v": "f",
                      "l_out": 0}]
            stores = list(tab_phase("x", None, "f1", 0))
            if scope in ("tabag", "agg1", "f1out"):
                allgather_table(stores, 0, 0)
            if scope in ("agg1", "f1out"):
                agg_phase("f", "f1", 0, 0)

        # ---- pooled -> SE attention scalars ----
        pool_red = singles.tile([128, 6], f32)
        nc.gpsimd.partition_all_reduce(pool_red[:], pooled_acc[:], 128,
                                       bass_isa.ReduceOp.add)
        nc.sync.dma_start(out=pool6_in[:], in_=pool_red[0:1, 0:6])
        nc.gpsimd.collective_compute(
            "AllReduce", mybir.AluOpType.add,
            replica_groups=[list(range(NCORES))],
            ins=[pool6_in[:]], outs=[pool6_out[:]],
        )
        pvec2 = singles.tile([6, 1], f32)
        nc.sync.dma_start(out=pvec2[:], in_=pool6_out[:])
        corr_sb = singles.tile([6, 1], f32)
        nc.sync.dma_start(out=corr_sb[:], in_=corr_in.unsqueeze(1))
        # remove pad-column relu(bias) pollution, then mean
        nc.vector.tensor_tensor(out=pvec2[:], in0=pvec2[:], in1=corr_sb[:],
                                op=mybir.AluOpType.subtract)
        nc.vector.tensor_scalar_mul(pvec2[:], pvec2[:], 1.0 / (N * FM))
        fc1wT = singles.tile([6, 30], f32)
        nc.sync.dma_start(out=fc1wT[:], in_=fc1wT_in[:])
        fc1b = singles.tile([30, 1], f32)
        nc.sync.dma_start(out=fc1b[:], in_=fc1b_in.unsqueeze(1))
        fc2wT = singles.tile([30, 6], f32)
        nc.sync.dma_start(out=fc2wT[:], in_=fc2wT_in[:])
        fc2b = singles.tile([6, 1], f32)
        nc.sync.dma_start(out=fc2b[:], in_=fc2b_in.unsqueeze(1))
        pz1 = psB.tile([30, 1], f32, tag="tabps")
        nc.tensor.matmul(pz1[:], lhsT=fc1wT[:], rhs=pvec2[:], start=True, stop=True)
        z1 = singles.tile([30, 1], f32)
        nc.vector.tensor_tensor(out=z1[:], in0=pz1[:], in1=fc1b[:], op=mybir.AluOpType.add)
        nc.vector.tensor_scalar_max(z1[:], z1[:], 0.0)
        pz2 = psB.tile([6, 1], f32, tag="tabps")
        nc.tensor.matmul(pz2[:], lhsT=fc2wT[:], rhs=z1[:], start=True, stop=True)
        z2 = singles.tile([6, 1], f32)
        nc.vector.tensor_tensor(out=z2[:], in0=pz2[:], in1=fc2b[:], op=mybir.AluOpType.add)
        av = singles.tile([6, 1], f32)
        nc.scalar.activation(out=av[:], in_=z2[:], func=mybir.ActivationFunctionType.Sigmoid)
        nc.sync.dma_start(out=a_scr[:], in_=av[:, 0])
        a_b = singles.tile([128, 6], f32)
        nc.gpsimd.dma_start(out=a_b[:], in_=a_scr[:].partition_broadcast(128))
        cnnw_b = singles.tile([128, 6], f32)
        nc.gpsimd.dma_start(out=cnnw_b[:], in_=cnnw_in.partition_broadcast(128))
        cnnb_b = singles.tile([128, 1], f32)
        nc.gpsimd.dma_start(out=cnnb_b[:], in_=cnnb_in.partition_broadcast(128))

        # ---- final combine: outT = sum_l cnnw_l * relu(a_l * fT_l) + cnn_b ----
        if scope == "f1out":
            for p in range(NPOS):
                cols = slice(p * 128, (p + 1) * 128)
                fl0 = fpo.tile([128, 128], bf16, tag="fin", name=f"fl0_{p}")
                nc.sync.dma_start(out=fl0[:], in_=fT_sl[0][:, cols])
                fo = fpo.tile([128, 128], f32, tag="ftmp", name=f"fo_{p}")
                nc.vector.tensor_copy(out=fo[:], in_=fl0[:])
                nc.sync.dma_start(out=out_d[:, cols], in_=fo[:])
        nlayers = 6 if scope == "full" else 1
        for p in range(NPOS) if scope != "f1out" else []:
            cols = slice(p * 128, (p + 1) * 128)
            acc = fpo.tile([128, 128], f32, tag="facc")
            for l in range(nlayers):
                fl = fpo.tile([128, 128], bf16, tag="fin")
                nc.sync.dma_start(out=fl[:], in_=fT_sl[l][:, cols])
                t = fpo.tile([128, 128], f32, tag="ftmp")
                nc.scalar.activation(out=t[:], in_=fl[:],
                                     func=mybir.ActivationFunctionType.Relu,
                                     scale=a_b[:, l:l + 1])
                if l == 0:
                    nc.vector.tensor_scalar_mul(acc[:], t[:], cnnw_b[:, 0:1])
                else:
                    nc.vector.tensor_scalar_mul(t[:], t[:], cnnw_b[:, l:l + 1])
                    nc.vector.tensor_tensor(out=acc[:], in0=acc[:], in1=t[:],
                                            op=mybir.AluOpType.add)
            nc.vector.tensor_scalar_add(acc[:], acc[:], cnnb_b[:, 0:1])
            nc.sync.dma_start(out=out_d[:, cols], in_=acc[:])

    nc.compile()
    _split_multiwaits(nc)
    return nc


def kernel(**inputs):
    global _last_exec_time_ns
    inputs = {k: np.asarray(v) for k, v in inputs.items()}

    meta = {}
    perview = {}
    for v in VIEWS:
        idx_arrs, sel_arrs, NLO, NHI = _prep_view(
            inputs[f"edges_{v}"].astype(np.int64), inputs[f"ew_{v}"])
        meta[v] = (NLO, NHI, int((NLO + NHI).sum()))
        perview[v] = (idx_arrs, sel_arrs)

    nc = _build(meta)

    xT = inputs["x_m"].T.astype(np.float32)  # [128, N]
    xT_pad = np.zeros((128, NPAD), np.float32)
    xT_pad[:, :N] = xT
    xT_pad = xT_pad.astype(bfnp)
    ident_np = np.eye(128, dtype=bfnp)
    # pad dst columns (node ids >= N, all on core 7) read relu(bias) into the
    # pooled sum; precompute the exact pollution per layer and subtract it.
    npad_cols = NPAD - N
    corr = np.array(
        [npad_cols * np.maximum(inputs[f"b_{nm}"].astype(np.float64), 0).sum()
         for nm in ["f1", "f2", "s1", "s2", "g1", "g2"]], np.float32)

    in_maps = []
    for c in range(NCORES):
        m = {
            "xT_slice": np.ascontiguousarray(xT_pad[:, c * RPC:(c + 1) * RPC]),
            "ident": ident_np,
            "fc1wT": inputs["fc1_w"].T.astype(np.float32).copy(),
            "fc1b": inputs["fc1_b"].astype(np.float32),
            "fc2wT": inputs["fc2_w"].T.astype(np.float32).copy(),
            "fc2b": inputs["fc2_b"].astype(np.float32),
            "cnnw": inputs["cnn_w"].astype(np.float32),
            "cnnb": inputs["cnn_b"].astype(np.float32),
            "corr": corr,
        }
        for nm in ["f1", "f2", "s1", "s2", "g1", "g2"]:
            m[f"W_{nm}"] = inputs[f"W_{nm}"].astype(bfnp)
            m[f"b_{nm}"] = inputs[f"b_{nm}"].astype(np.float32)
        for v in VIEWS:
            idx_arrs, sel_arrs = perview[v]
            m[f"idx_{v}"] = idx_arrs[c]
            m[f"sel_{v}"] = sel_arrs[c]
        in_maps.append(m)

    trace = os.environ.get("KERNEL_TRACE", "0") == "1"
    kw = {}
    if trace:
        td = os.environ.get("KERNEL_TRACE_DIR")
        if td:
            os.makedirs(td, exist_ok=True)
            kw["tmpdir"] = td
    res = run_bass_kernel_spmd(nc, in_maps, list(range(NCORES)), trace=trace, **kw)
    _last_exec_time_ns = res.exec_time_ns
    outT = np.concatenate([res.results[c]["out_slice"] for c in range(NCORES)], axis=1)
    return np.ascontiguousarray(outT.T[:N]).astype(np.float32)
